# revision 12
# baseline (speedup 1.0000x reference)
"""Trainium2 Bass kernel for nn_AtBatCell: GRU recurrence over a shared state
table with gather/scatter-add per timestep.

Strategy: steps touching disjoint table rows are independent, so the T=8192
sequential scan collapses into ~6 "waves" (levels of the row-dependency DAG).
Each wave is a large batch of independent GRU cell applications.

Key structure (per 128-step chunk, batched in 4-chunk blocks):
 - rows whose FIRST touch is in wave 1 (i.e. all of wave 1) are host-packed
   into a contiguous stream -> plain DMA, no per-row descriptors
 - other rows come via dma_gather (Q7 SWDGE MoE gather)
 - matmuls run in bf16 on the PE (f32 PSUM accumulate); H / r*h transposes
   use the DMA xbar (dma_start_transpose) instead of the PE
 - deltas (dh) are shipped to DRAM contiguously; the host applies them with
   ordered np.add.at. Device scatter-adds only rows that a LATER wave will
   gather again (~20% of touches).
"""
import sys
sys.path.insert(0, '/opt/trn_rl_repo')

import numpy as np

SIT = 64
S = 256
S2 = 512
CHUNK = 128          # steps per compute chunk
SPARE = 128          # spare zero rows absorbing padding/dup scatters
BLOCK = 4            # chunks per gather/scatter/dh block


def _schedule(b, p, n_rows_total):
    T = len(b)
    last_level = np.zeros(n_rows_total, dtype=np.int64)
    levels = np.empty(T, dtype=np.int64)
    bl = b.astype(np.int64)
    pl = p.astype(np.int64)
    for t in range(T):
        lv = max(last_level[bl[t]], last_level[pl[t]]) + 1
        levels[t] = lv
        last_level[bl[t]] = lv
        last_level[pl[t]] = lv
    n_waves = int(levels.max())
    order = np.argsort(levels, kind='stable')
    wave_sizes = np.bincount(levels, minlength=n_waves + 1)[1:]
    touched = np.unique(np.concatenate([bl, pl]))
    remap = np.full(n_rows_total, -1, dtype=np.int64)
    remap[touched] = np.arange(len(touched))
    return dict(levels=levels, order=order, wave_sizes=wave_sizes,
                n_waves=n_waves, touched=touched, remap=remap)


def _build_host_data(x, b, p, Wz, Wr, Wh, Uz, Ur, Uh, bz, br, bh, table0):
    import ml_dtypes
    bf16 = ml_dtypes.bfloat16
    N = table0.shape[0]
    b = b.astype(np.int64)
    p = p.astype(np.int64)
    sch = _schedule(b, p, N)
    order, wave_sizes = sch['order'], sch['wave_sizes']
    touched, remap = sch['touched'], sch['remap']
    n_real = len(touched)
    n_rows_c = n_real + SPARE

    T = len(b)
    pos = np.empty(T, np.int64)
    pos[order] = np.arange(T)

    # fresh (first touch of row) / keep (row touched again later), per slot
    import collections
    rowpos = collections.defaultdict(list)
    for t in range(T):
        rowpos[b[t]].append((pos[t], t, 0))
        rowpos[p[t]].append((pos[t], t, 1))
    fresh = np.zeros((T, 2), bool)
    keep = np.zeros((T, 2), bool)
    for r, lst in rowpos.items():
        lst.sort()
        fresh[lst[0][1], lst[0][2]] = True
        for (_, t, s) in lst[:-1]:
            keep[t, s] = True
    step_fresh = fresh.all(1)
    step_keep = keep.any(1)

    wave_chunks = [int(-(-int(ws) // CHUNK)) for ws in wave_sizes]
    n_chunks = sum(wave_chunks)
    T_pad = n_chunks * CHUNK

    b_s = np.full(T_pad, -1, dtype=np.int64)
    p_s = np.full(T_pad, -1, dtype=np.int64)
    x_s = np.zeros((T_pad, SIT), dtype=np.float32)
    bias_col = np.zeros(T_pad, dtype=np.float32)
    fresh_s = np.zeros(T_pad, bool)
    keep_s = np.zeros(T_pad, bool)
    posn = 0
    src = 0
    for w, ws in enumerate(wave_sizes):
        ws = int(ws)
        idxs = order[src:src + ws]
        src += ws
        # sort within wave: keep-steps first (so trailing blocks skip scatter)
        sk = np.argsort(~step_keep[idxs], kind='stable')
        idxs = idxs[sk]
        b_s[posn:posn + ws] = remap[b[idxs]]
        p_s[posn:posn + ws] = remap[p[idxs]]
        x_s[posn:posn + ws] = x[idxs]
        bias_col[posn:posn + ws] = 1.0
        fresh_s[posn:posn + ws] = step_fresh[idxs]
        keep_s[posn:posn + ws] = step_keep[idxs]
        posn += -(-ws // CHUNK) * CHUNK

    # padding -> spare rows; pads count as "fresh" (zero rows in stream)
    spare_ids = n_real + np.arange(SPARE)
    pad_pos = np.nonzero(b_s < 0)[0]
    b_s[pad_pos] = spare_ids[pad_pos % SPARE]
    p_s[pad_pos] = spare_ids[(pad_pos + 1) % SPARE]
    fresh_s[pad_pos] = True

    dup_steps = np.nonzero(b_s == p_s)[0]
    dupmask = np.zeros(T_pad, dtype=np.float32)
    if len(dup_steps):
        dupmask[dup_steps] = 1.0
        p_s[dup_steps] = spare_ids[dup_steps % SPARE]

    # interleaved idx order per chunk: [b_0..127 | p_0..127]
    bi = b_s.reshape(n_chunks, CHUNK)
    pi = p_s.reshape(n_chunks, CHUNK)
    idx_il = np.stack([bi, pi], axis=1).reshape(-1).astype(np.int16)
    n_idx = 2 * T_pad
    idx_wrapped = idx_il.reshape(n_idx // 16, 16).T
    idx_rep = np.tile(idx_wrapped, (8, 1)).copy()    # [128, n/16]

    table_c = np.zeros((n_rows_c, S), dtype=np.float32)
    table_c[:n_real] = table0[touched]

    # ---- block structure ----
    # blocks: per wave, runs of up to BLOCK chunks
    blocks = []   # (c0, nb, wave, all_fresh, any_keep, fresh_off or -1)
    fresh_cols = 0
    c0 = 0
    for w, wc in enumerate(wave_chunks):
        for bstart in range(0, wc, BLOCK):
            nb = min(BLOCK, wc - bstart)
            cs = c0 + bstart
            sl = slice(cs * CHUNK, (cs + nb) * CHUNK)
            af = bool(fresh_s[sl].all())
            ak = bool(keep_s[sl].any())
            fo = -1
            if af:
                fo = fresh_cols
                fresh_cols += 2 * nb
            blocks.append((cs, nb, w, af, ak, fo))
        c0 += wc

    # fresh stream [128, fresh_cols, 256]: block at fo, chunk q, slot s ->
    # col fo + 2q + s, partition = step-in-chunk
    fresh_arr = np.zeros((128, max(fresh_cols, 2), S), dtype=np.float32)
    for (cs, nb, w, af, ak, fo) in blocks:
        if fo < 0:
            continue
        for q in range(nb):
            c = cs + q
            fresh_arr[:, fo + 2 * q, :] = table_c[b_s[c * CHUNK:(c + 1) * CHUNK]]
            fresh_arr[:, fo + 2 * q + 1, :] = table_c[p_s[c * CHUNK:(c + 1) * CHUNK]]

    # xT augmented with bias row (65, T_pad), bf16
    xT = np.zeros((SIT + 1, T_pad), dtype=np.float32)
    xT[:SIT] = x_s.T
    xT[SIT] = bias_col
    WzT = np.concatenate([Wz.T, bz[None, :]], axis=0)
    WrT = np.concatenate([Wr.T, -br[None, :]], axis=0)
    WhT = np.concatenate([Wh.T, bh[None, :]], axis=0)

    def ut(U):
        return np.ascontiguousarray(U.T.reshape(4, 128, S2).transpose(1, 0, 2))

    hd = dict(
        table_c=table_c, idx_rep=idx_rep, fresh_arr=fresh_arr,
        xT=xT.astype(bf16), WzT=WzT.astype(bf16), WrT=WrT.astype(bf16),
        WhT=WhT.astype(bf16), UzT=ut(Uz).astype(bf16), UrT=ut(Ur).astype(bf16),
        UhT=ut(Uh).astype(bf16),
        n_chunks=n_chunks, wave_chunks=wave_chunks, blocks=blocks,
        fresh_cols=max(fresh_cols, 2),
        n_rows_c=n_rows_c, n_real=n_real, touched=touched,
        b_s=b_s, p_s=p_s, dupmask=dupmask,
        dup_any=bool(len(dup_steps)), T_pad=T_pad,
    )
    return hd


def _build_nc(hd):
    import concourse.bacc as bacc
    import concourse.mybir as mybir
    import concourse.tile as tile
    from concourse.masks import make_identity

    n_rows_c = hd['n_rows_c']
    n_chunks = hd['n_chunks']
    T_pad = hd['T_pad']
    blocks = hd['blocks']
    f32 = mybir.dt.float32
    bf16 = mybir.dt.bfloat16
    i16 = mybir.dt.int16

    nc = bacc.Bacc("TRN2", target_bir_lowering=False, debug=True)

    tab_in = nc.dram_tensor("table", (n_rows_c, S), f32, kind="ExternalInput")
    idx_in = nc.dram_tensor("idx", (128, 2 * T_pad // 16), i16, kind="ExternalInput")
    fresh_in = nc.dram_tensor("fresh", (128, hd['fresh_cols'], S), f32,
                              kind="ExternalInput")
    xT_in = nc.dram_tensor("xT", (SIT + 1, T_pad), bf16, kind="ExternalInput")
    WzT_in = nc.dram_tensor("WzT", (SIT + 1, S2), bf16, kind="ExternalInput")
    WrT_in = nc.dram_tensor("WrT", (SIT + 1, S2), bf16, kind="ExternalInput")
    WhT_in = nc.dram_tensor("WhT", (SIT + 1, S2), bf16, kind="ExternalInput")
    UzT_in = nc.dram_tensor("UzT", (128, 4, S2), bf16, kind="ExternalInput")
    UrT_in = nc.dram_tensor("UrT", (128, 4, S2), bf16, kind="ExternalInput")
    UhT_in = nc.dram_tensor("UhT", (128, 4, S2), bf16, kind="ExternalInput")
    dmask_in = nc.dram_tensor("dmask", (128, n_chunks), f32, kind="ExternalInput")

    dh_out = nc.dram_tensor("dh", (128, 2 * n_chunks, S), f32,
                            kind="ExternalOutput")
    tab_work = nc.dram_tensor("tabw", (n_rows_c, S), f32)  # internal scratch

    Sig = mybir.ActivationFunctionType.Sigmoid
    Tanh = mybir.ActivationFunctionType.Tanh

    with tile.TileContext(nc) as tc:
        with tc.tile_pool(name="const", bufs=1) as cpool, \
             tc.tile_pool(name="gath", bufs=6) as gpool, \
             tc.tile_pool(name="dhb", bufs=6) as dhpool, \
             tc.tile_pool(name="work", bufs=3) as wpool, \
             tc.tile_pool(name="psA", bufs=2, space="PSUM") as psA, \
             tc.tile_pool(name="psZ", bufs=2, space="PSUM") as psZ, \
             tc.tile_pool(name="psR", bufs=2, space="PSUM") as psR, \
             tc.tile_pool(name="psM", bufs=2, space="PSUM") as psM:

            # ---- static loads (sync HWDGE) ----
            idx_sb = cpool.tile([128, 2 * T_pad // 16], i16, tag="idx")
            nc.sync.dma_start(idx_sb[:], idx_in[:])
            xT_sb = cpool.tile([SIT + 1, T_pad], bf16, tag="xT")
            nc.sync.dma_start(xT_sb[:], xT_in[:])
            w_sb = {}
            for nm, t in (("WzT", WzT_in), ("WrT", WrT_in), ("WhT", WhT_in)):
                w_sb[nm] = cpool.tile([SIT + 1, S2], bf16, tag=nm, name=nm + "_sb")
                nc.sync.dma_start(w_sb[nm][:], t[:])
            for nm, t in (("UzT", UzT_in), ("UrT", UrT_in), ("UhT", UhT_in)):
                w_sb[nm] = cpool.tile([128, 4, S2], bf16, tag=nm, name=nm + "_sb")
                nc.sync.dma_start(w_sb[nm][:], t[:])
            dmask_sb = cpool.tile([128, n_chunks], f32, tag="dmask")
            if hd['dup_any']:
                nc.sync.dma_start(dmask_sb[:], dmask_in[:])
            ident = cpool.tile([128, 128], f32, tag="ident")
            make_identity(nc, ident[:])
            identb = cpool.tile([128, 128], bf16, tag="identb")
            make_identity(nc, identb[:])

            copied = False

            def emit_copy():
                # init copy of the table scratch (SWDGE so it doesn't block
                # the sync HWDGE ring); sliced: one 13MB D2D DMA faults
                CP = 1024
                for r0 in range(0, n_rows_c, CP):
                    r1 = min(r0 + CP, n_rows_c)
                    nc.gpsimd.dma_start(tab_work[r0:r1, :], tab_in[r0:r1, :])

            def emit_gather(blk):
                (cs2, nb2, _, af2, ak2, fo2) = blk
                g = gpool.tile([128, 2 * BLOCK, S], f32, tag="hg",
                               name=f"hg_{cs2}")
                if af2:
                    nc.sync.dma_start(
                        g[:, 0:2 * nb2, :],
                        fresh_in[:, fo2:fo2 + 2 * nb2, :])
                else:
                    nc.gpsimd.dma_gather(
                        out_ap=g[:, 0:2 * nb2, :], in_ap=tab_work[:],
                        idxs_ap=idx_sb[:, 16 * cs2:16 * (cs2 + nb2)],
                        num_idxs=2 * CHUNK * nb2,
                        num_idxs_reg=2 * CHUNK * nb2,
                        elem_size=S, queue_num=0,
                    )
                return g

            PREFETCH = 3
            cur_wave = -1
            for (cs, nb, w, all_fresh, any_keep, fo) in blocks:
                if w != cur_wave:
                    cur_wave = w
                    wave_blocks = [blk for blk in blocks if blk[2] == w]
                    gtiles = {}
                    wave_fresh = all(blk[3] for blk in wave_blocks)
                    if wave_fresh:
                        # rolling prefetch (no tab_work reads -> interleaving
                        # with scatters is safe)
                        for blk in wave_blocks[:PREFETCH]:
                            gtiles[blk[0]] = emit_gather(blk)
                        pending = wave_blocks[PREFETCH:]
                    else:
                        # dma_gather reads tab_work: all reads must precede
                        # this wave's scatters in emission order
                        for blk in wave_blocks:
                            gtiles[blk[0]] = emit_gather(blk)
                        pending = []
                    if not copied:
                        copied = True
                        emit_copy()

                g = gtiles.pop(cs)
                if pending:
                    blk = pending.pop(0)
                    gtiles[blk[0]] = emit_gather(blk)
                dhb = dhpool.tile([128, 2 * BLOCK, S], f32, tag="dh",
                                  name=f"dh_{cs}")
                for q in range(nb):
                    c = cs + q
                    hg2 = g[:, 2 * q:2 * q + 2, :].rearrange("p a b -> p (a b)")

                    # PE transpose of H in f32; the PSUM->SBUF copy casts bf16
                    ht_ps = psA.tile([128, 4, CHUNK], f32, tag="tr")
                    for k in range(4):
                        nc.tensor.transpose(
                            ht_ps[:, k, :], hg2[:, CHUNK * k:CHUNK * (k + 1)],
                            ident[:])
                    ht = wpool.tile([128, 4, CHUNK], bf16, tag="ht")
                    nc.vector.tensor_copy(ht[:], ht_ps[:])

                    xt_c = xT_sb[:, CHUNK * c:CHUNK * (c + 1)]

                    zpre = psZ.tile([128, S2], f32, tag="zpre")
                    rpre = psR.tile([128, S2], f32, tag="rpre")
                    nc.tensor.matmul(zpre[:], xt_c, w_sb["WzT"][:],
                                     start=True, stop=False)
                    for k in range(4):
                        nc.tensor.matmul(zpre[:], ht[:, k, :], w_sb["UzT"][:, k, :],
                                         start=False, stop=(k == 3))
                    nc.tensor.matmul(rpre[:], xt_c, w_sb["WrT"][:],
                                     start=True, stop=False)
                    for k in range(4):
                        nc.tensor.matmul(rpre[:], ht[:, k, :], w_sb["UrT"][:, k, :],
                                         start=False, stop=(k == 3))

                    zc = wpool.tile([128, S2], f32, tag="zc")
                    r = wpool.tile([128, S2], f32, tag="r")
                    nc.scalar.activation(zc[:], zpre[:], Sig, scale=-1.0)  # 1-z
                    nc.scalar.activation(r[:], rpre[:], Sig)

                    rh = wpool.tile([128, S2], bf16, tag="rh")
                    nc.vector.tensor_mul(rh[:], r[:], hg2)
                    rht_ps_f = psA.tile([128, 4, CHUNK], f32, tag="tr",
                                        name=f"rhtp_{c}")
                    rht_ps = rht_ps_f[:].bitcast(bf16)[:, :, 0:CHUNK]
                    for k in range(4):
                        nc.tensor.transpose(
                            rht_ps[:, k, :], rh[:, CHUNK * k:CHUNK * (k + 1)],
                            identb[:])
                    rht = wpool.tile([128, 4, CHUNK], bf16, tag="rht")
                    nc.vector.tensor_copy(rht[:], rht_ps)

                    mpre = psM.tile([128, S2], f32, tag="mpre")
                    nc.tensor.matmul(mpre[:], xt_c, w_sb["WhT"][:],
                                     start=True, stop=False)
                    for k in range(4):
                        nc.tensor.matmul(mpre[:], rht[:, k, :], w_sb["UhT"][:, k, :],
                                         start=False, stop=(k == 3))

                    m = wpool.tile([128, S2], f32, tag="m")
                    nc.scalar.activation(m[:], mpre[:], Tanh)

                    # dh = (1-z)*(m-h)
                    t1 = wpool.tile([128, S2], f32, tag="t1")
                    nc.vector.tensor_sub(t1[:], m[:], hg2)
                    dh_view = dhb[:, 2 * q:2 * (q + 1), :].rearrange(
                        "p a b -> p (a b)")
                    nc.vector.tensor_mul(dh_view, zc[:], t1[:])
                    if hd['dup_any']:
                        tm = wpool.tile([128, S], f32, tag="tm")
                        nc.vector.tensor_scalar_mul(
                            tm[:], dhb[:, 2 * q + 1, :], dmask_sb[:, c:c + 1])
                        nc.vector.tensor_add(
                            dhb[:, 2 * q, :], dhb[:, 2 * q, :], tm[:])

                # ship deltas to host (sync HWDGE)
                nc.sync.dma_start(dh_out[:, 2 * cs:2 * (cs + nb), :],
                                  dhb[:, 0:2 * nb, :])
                if any_keep:
                    nidx = 2 * CHUNK * nb
                    nc.gpsimd.dma_scatter_add(
                        tab_work[:], dhb[:, 0:2 * nb, :],
                        idx_sb[:, 16 * cs:16 * cs + nidx // 16],
                        nidx, nidx, S, queue_num=0,
                    )

    nc.compile()
    return nc


def _in_map(hd):
    dmask_full = np.zeros((128, hd['n_chunks']), dtype=np.float32)
    dmask_full[:] = hd['dupmask'].reshape(hd['n_chunks'], CHUNK).T
    return {
        "table": hd['table_c'], "idx": hd['idx_rep'], "fresh": hd['fresh_arr'],
        "xT": hd['xT'], "WzT": hd['WzT'], "WrT": hd['WrT'], "WhT": hd['WhT'],
        "UzT": hd['UzT'], "UrT": hd['UrT'], "UhT": hd['UhT'],
        "dmask": dmask_full,
    }


def _run(hd, nc, trace=False):
    from concourse.bass_utils import run_bass_kernel_spmd
    in_map = _in_map(hd)
    return run_bass_kernel_spmd(nc, [dict(in_map) for _ in range(8)],
                                list(range(8)), trace=trace)


def _assemble(hd, dh_res, table0):
    """Host-side final assembly: ordered scatter-add of all deltas."""
    n_chunks = hd['n_chunks']
    # dh_res [128, 2*n_chunks, 256] -> per step i=c*128+p, slot s: [p, 2c+s, :]
    dh = np.ascontiguousarray(dh_res.transpose(1, 0, 2))  # [2c+s, p, 256]
    dh = dh.reshape(n_chunks, 2, CHUNK, S).transpose(0, 2, 1, 3)
    dh = dh.reshape(hd['T_pad'] * 2, S)  # [(step, slot), 256] schedule order
    rows = np.stack([hd['b_s'], hd['p_s']], axis=1).reshape(-1)
    acc = hd['table_c'].copy()
    np.add.at(acc, rows, dh)
    out = table0.copy()
    out[hd['touched']] = acc[:hd['n_real']]
    return out


def kernel(**inputs):
    x = np.asarray(inputs['x'], dtype=np.float32)
    b = np.asarray(inputs['b'])
    p = np.asarray(inputs['p'])
    table0 = np.asarray(inputs['table0'], dtype=np.float32)

    hd = _build_host_data(
        x, b, p,
        np.asarray(inputs['Wz'], np.float32), np.asarray(inputs['Wr'], np.float32),
        np.asarray(inputs['Wh'], np.float32), np.asarray(inputs['Uz'], np.float32),
        np.asarray(inputs['Ur'], np.float32), np.asarray(inputs['Uh'], np.float32),
        np.asarray(inputs['bz'], np.float32), np.asarray(inputs['br'], np.float32),
        np.asarray(inputs['bh'], np.float32), table0)

    nc = _build_nc(hd)
    res = _run(hd, nc)
    return _assemble(hd, np.asarray(res.results[0]["dh"], np.float32), table0)


if __name__ == "__main__":
    d = np.load('/tmp/ref_inputs.npz')
    inputs = {k: d[k] for k in d.files}
    got = kernel(**inputs)
    exp = np.load('/tmp/ref_out_np.npy')
    err = np.abs(got - exp).max()
    print("abs err:", err, "rel:", err / np.abs(exp).max())


# revision 14
# speedup vs baseline: 1.1116x; 1.1116x over previous
"""Trainium2 Bass kernel for nn_AtBatCell: GRU recurrence over a shared state
table with gather/scatter-add per timestep.

Strategy: steps touching disjoint table rows are independent, so the T=8192
sequential scan collapses into ~6 "waves" (levels of the row-dependency DAG).
Each wave is a large batch of independent GRU cell applications.

Key structure (per 128-step chunk, batched in 4-chunk blocks):
 - rows whose FIRST touch is in wave 1 (i.e. all of wave 1) are host-packed
   into a contiguous stream -> plain DMA, no per-row descriptors
 - other rows come via dma_gather (Q7 SWDGE MoE gather)
 - matmuls run in bf16 on the PE (f32 PSUM accumulate); H / r*h transposes
   use the DMA xbar (dma_start_transpose) instead of the PE
 - deltas (dh) are shipped to DRAM contiguously; the host applies them with
   ordered np.add.at. Device scatter-adds only rows that a LATER wave will
   gather again (~20% of touches).
"""
import os
import sys
for _p in ('/opt/trn_rl_repo', '/root/.axon_site/_ro/trn_rl_repo'):
    if os.path.isdir(_p) and _p not in sys.path:
        sys.path.insert(0, _p)

import numpy as np

SIT = 64
S = 256
S2 = 512
CHUNK = 128          # steps per compute chunk
SPARE = 128          # spare zero rows absorbing padding/dup scatters
BLOCK = 4            # chunks per gather/scatter/dh block


def _schedule(b, p, n_rows_total):
    T = len(b)
    last_level = np.zeros(n_rows_total, dtype=np.int64)
    levels = np.empty(T, dtype=np.int64)
    bl = b.astype(np.int64)
    pl = p.astype(np.int64)
    for t in range(T):
        lv = max(last_level[bl[t]], last_level[pl[t]]) + 1
        levels[t] = lv
        last_level[bl[t]] = lv
        last_level[pl[t]] = lv
    n_waves = int(levels.max())
    order = np.argsort(levels, kind='stable')
    wave_sizes = np.bincount(levels, minlength=n_waves + 1)[1:]
    touched = np.unique(np.concatenate([bl, pl]))
    remap = np.full(n_rows_total, -1, dtype=np.int64)
    remap[touched] = np.arange(len(touched))
    return dict(levels=levels, order=order, wave_sizes=wave_sizes,
                n_waves=n_waves, touched=touched, remap=remap)


def _build_host_data(x, b, p, Wz, Wr, Wh, Uz, Ur, Uh, bz, br, bh, table0):
    import ml_dtypes
    bf16 = ml_dtypes.bfloat16
    N = table0.shape[0]
    b = b.astype(np.int64)
    p = p.astype(np.int64)
    sch = _schedule(b, p, N)
    order, wave_sizes = sch['order'], sch['wave_sizes']
    touched, remap = sch['touched'], sch['remap']
    n_real = len(touched)
    n_rows_c = n_real + SPARE

    T = len(b)
    pos = np.empty(T, np.int64)
    pos[order] = np.arange(T)

    # fresh (first touch of row) / keep (row touched again later), per slot
    import collections
    rowpos = collections.defaultdict(list)
    for t in range(T):
        rowpos[b[t]].append((pos[t], t, 0))
        rowpos[p[t]].append((pos[t], t, 1))
    fresh = np.zeros((T, 2), bool)
    keep = np.zeros((T, 2), bool)
    for r, lst in rowpos.items():
        lst.sort()
        fresh[lst[0][1], lst[0][2]] = True
        for (_, t, s) in lst[:-1]:
            keep[t, s] = True
    step_fresh = fresh.all(1)
    step_keep = keep.any(1)

    wave_chunks = [int(-(-int(ws) // CHUNK)) for ws in wave_sizes]
    n_chunks = sum(wave_chunks)
    T_pad = n_chunks * CHUNK

    b_s = np.full(T_pad, -1, dtype=np.int64)
    p_s = np.full(T_pad, -1, dtype=np.int64)
    x_s = np.zeros((T_pad, SIT), dtype=np.float32)
    bias_col = np.zeros(T_pad, dtype=np.float32)
    fresh_s = np.zeros(T_pad, bool)
    keep_s = np.zeros(T_pad, bool)
    posn = 0
    src = 0
    for w, ws in enumerate(wave_sizes):
        ws = int(ws)
        idxs = order[src:src + ws]
        src += ws
        # sort within wave: keep-steps first (so trailing blocks skip scatter)
        sk = np.argsort(~step_keep[idxs], kind='stable')
        idxs = idxs[sk]
        b_s[posn:posn + ws] = remap[b[idxs]]
        p_s[posn:posn + ws] = remap[p[idxs]]
        x_s[posn:posn + ws] = x[idxs]
        bias_col[posn:posn + ws] = 1.0
        fresh_s[posn:posn + ws] = step_fresh[idxs]
        keep_s[posn:posn + ws] = step_keep[idxs]
        posn += -(-ws // CHUNK) * CHUNK

    # padding -> spare rows; pads count as "fresh" (zero rows in stream)
    spare_ids = n_real + np.arange(SPARE)
    pad_pos = np.nonzero(b_s < 0)[0]
    b_s[pad_pos] = spare_ids[pad_pos % SPARE]
    p_s[pad_pos] = spare_ids[(pad_pos + 1) % SPARE]
    fresh_s[pad_pos] = True

    dup_steps = np.nonzero(b_s == p_s)[0]
    dupmask = np.zeros(T_pad, dtype=np.float32)
    if len(dup_steps):
        dupmask[dup_steps] = 1.0
        p_s[dup_steps] = spare_ids[dup_steps % SPARE]

    # interleaved idx order per chunk: [b_0..127 | p_0..127]
    bi = b_s.reshape(n_chunks, CHUNK)
    pi = p_s.reshape(n_chunks, CHUNK)
    idx_il = np.stack([bi, pi], axis=1).reshape(-1).astype(np.int16)
    n_idx = 2 * T_pad
    idx_wrapped = idx_il.reshape(n_idx // 16, 16).T
    idx_rep = np.tile(idx_wrapped, (8, 1)).copy()    # [128, n/16]

    table_c = np.zeros((n_rows_c, S), dtype=np.float32)
    table_c[:n_real] = table0[touched]

    # ---- block structure ----
    # blocks: per wave, runs of up to BLOCK chunks
    blocks = []   # (c0, nb, wave, all_fresh, any_keep, fresh_off or -1)
    fresh_cols = 0
    c0 = 0
    for w, wc in enumerate(wave_chunks):
        for bstart in range(0, wc, BLOCK):
            nb = min(BLOCK, wc - bstart)
            cs = c0 + bstart
            sl = slice(cs * CHUNK, (cs + nb) * CHUNK)
            af = bool(fresh_s[sl].all())
            ak = bool(keep_s[sl].any())
            fo = -1
            if af:
                fo = fresh_cols
                fresh_cols += 2 * nb
            blocks.append((cs, nb, w, af, ak, fo))
        c0 += wc

    # fresh stream [128, fresh_cols, 256]: block at fo, chunk q, slot s ->
    # col fo + 2q + s, partition = step-in-chunk
    fresh_arr = np.zeros((128, max(fresh_cols, 2), S), dtype=np.float32)
    for (cs, nb, w, af, ak, fo) in blocks:
        if fo < 0:
            continue
        for q in range(nb):
            c = cs + q
            fresh_arr[:, fo + 2 * q, :] = table_c[b_s[c * CHUNK:(c + 1) * CHUNK]]
            fresh_arr[:, fo + 2 * q + 1, :] = table_c[p_s[c * CHUNK:(c + 1) * CHUNK]]

    # xT augmented with bias row (65, T_pad), bf16
    xT = np.zeros((SIT + 1, T_pad), dtype=np.float32)
    xT[:SIT] = x_s.T
    xT[SIT] = bias_col
    WzT = np.concatenate([Wz.T, bz[None, :]], axis=0)
    WrT = np.concatenate([Wr.T, -br[None, :]], axis=0)
    WhT = np.concatenate([Wh.T, bh[None, :]], axis=0)

    def ut(U):
        return np.ascontiguousarray(U.T.reshape(4, 128, S2).transpose(1, 0, 2))

    hd = dict(
        table_c=table_c, idx_rep=idx_rep, fresh_arr=fresh_arr,
        xT=xT.astype(bf16), WzT=WzT.astype(bf16), WrT=WrT.astype(bf16),
        WhT=WhT.astype(bf16), UzT=ut(Uz).astype(bf16), UrT=ut(Ur).astype(bf16),
        UhT=ut(Uh).astype(bf16),
        n_chunks=n_chunks, wave_chunks=wave_chunks, blocks=blocks,
        fresh_cols=max(fresh_cols, 2),
        n_rows_c=n_rows_c, n_real=n_real, touched=touched,
        b_s=b_s, p_s=p_s, dupmask=dupmask,
        dup_any=bool(len(dup_steps)), T_pad=T_pad,
    )
    return hd


def _build_nc(hd):
    import concourse.bacc as bacc
    import concourse.mybir as mybir
    import concourse.tile as tile
    from concourse.masks import make_identity

    n_rows_c = hd['n_rows_c']
    n_chunks = hd['n_chunks']
    T_pad = hd['T_pad']
    blocks = hd['blocks']
    f32 = mybir.dt.float32
    bf16 = mybir.dt.bfloat16
    i16 = mybir.dt.int16

    nc = bacc.Bacc("TRN2", target_bir_lowering=False, debug=True)

    tab_in = nc.dram_tensor("table", (n_rows_c, S), f32, kind="ExternalInput")
    idx_in = nc.dram_tensor("idx", (128, 2 * T_pad // 16), i16, kind="ExternalInput")
    fresh_in = nc.dram_tensor("fresh", (128, hd['fresh_cols'], S), f32,
                              kind="ExternalInput")
    xT_in = nc.dram_tensor("xT", (SIT + 1, T_pad), bf16, kind="ExternalInput")
    WzT_in = nc.dram_tensor("WzT", (SIT + 1, S2), bf16, kind="ExternalInput")
    WrT_in = nc.dram_tensor("WrT", (SIT + 1, S2), bf16, kind="ExternalInput")
    WhT_in = nc.dram_tensor("WhT", (SIT + 1, S2), bf16, kind="ExternalInput")
    UzT_in = nc.dram_tensor("UzT", (128, 4, S2), bf16, kind="ExternalInput")
    UrT_in = nc.dram_tensor("UrT", (128, 4, S2), bf16, kind="ExternalInput")
    UhT_in = nc.dram_tensor("UhT", (128, 4, S2), bf16, kind="ExternalInput")
    dmask_in = nc.dram_tensor("dmask", (128, n_chunks), f32, kind="ExternalInput")

    dh_out = nc.dram_tensor("dh", (128, 2 * n_chunks, S), f32,
                            kind="ExternalOutput")
    tab_work = nc.dram_tensor("tabw", (n_rows_c, S), f32)  # internal scratch

    Sig = mybir.ActivationFunctionType.Sigmoid
    Tanh = mybir.ActivationFunctionType.Tanh

    with tile.TileContext(nc) as tc:
        with tc.tile_pool(name="const", bufs=1) as cpool, \
             tc.tile_pool(name="gath", bufs=6) as gpool, \
             tc.tile_pool(name="dhb", bufs=6) as dhpool, \
             tc.tile_pool(name="work", bufs=3) as wpool, \
             tc.tile_pool(name="psA", bufs=2, space="PSUM") as psA, \
             tc.tile_pool(name="psZ", bufs=2, space="PSUM") as psZ, \
             tc.tile_pool(name="psR", bufs=2, space="PSUM") as psR, \
             tc.tile_pool(name="psM", bufs=2, space="PSUM") as psM:

            # ---- static loads (sync HWDGE) ----
            idx_sb = cpool.tile([128, 2 * T_pad // 16], i16, tag="idx")
            nc.sync.dma_start(idx_sb[:], idx_in[:])
            xT_sb = cpool.tile([SIT + 1, T_pad], bf16, tag="xT")
            nc.sync.dma_start(xT_sb[:], xT_in[:])
            w_sb = {}
            for nm, t in (("WzT", WzT_in), ("WrT", WrT_in), ("WhT", WhT_in)):
                w_sb[nm] = cpool.tile([SIT + 1, S2], bf16, tag=nm, name=nm + "_sb")
                nc.sync.dma_start(w_sb[nm][:], t[:])
            for nm, t in (("UzT", UzT_in), ("UrT", UrT_in), ("UhT", UhT_in)):
                w_sb[nm] = cpool.tile([128, 4, S2], bf16, tag=nm, name=nm + "_sb")
                nc.sync.dma_start(w_sb[nm][:], t[:])
            dmask_sb = cpool.tile([128, n_chunks], f32, tag="dmask")
            if hd['dup_any']:
                nc.sync.dma_start(dmask_sb[:], dmask_in[:])
            ident = cpool.tile([128, 128], f32, tag="ident")
            make_identity(nc, ident[:])
            identb = cpool.tile([128, 128], bf16, tag="identb")
            make_identity(nc, identb[:])

            copied = False

            def emit_copy():
                # init copy of the table scratch (SWDGE so it doesn't block
                # the sync HWDGE ring); sliced: one 13MB D2D DMA faults
                CP = 1024
                for r0 in range(0, n_rows_c, CP):
                    r1 = min(r0 + CP, n_rows_c)
                    nc.gpsimd.dma_start(tab_work[r0:r1, :], tab_in[r0:r1, :])

            def emit_gather(blk):
                (cs2, nb2, _, af2, ak2, fo2) = blk
                g = gpool.tile([128, 2 * BLOCK, S], f32, tag="hg",
                               name=f"hg_{cs2}")
                if af2:
                    nc.sync.dma_start(
                        g[:, 0:2 * nb2, :],
                        fresh_in[:, fo2:fo2 + 2 * nb2, :])
                else:
                    nc.gpsimd.dma_gather(
                        out_ap=g[:, 0:2 * nb2, :], in_ap=tab_work[:],
                        idxs_ap=idx_sb[:, 16 * cs2:16 * (cs2 + nb2)],
                        num_idxs=2 * CHUNK * nb2,
                        num_idxs_reg=2 * CHUNK * nb2,
                        elem_size=S, queue_num=0,
                    )
                return g

            PREFETCH = 4
            cur_wave = -1
            for (cs, nb, w, all_fresh, any_keep, fo) in blocks:
                if w != cur_wave:
                    cur_wave = w
                    wave_blocks = [blk for blk in blocks if blk[2] == w]
                    gtiles = {}
                    wave_fresh = all(blk[3] for blk in wave_blocks)
                    if wave_fresh:
                        # rolling prefetch (no tab_work reads -> interleaving
                        # with scatters is safe)
                        for blk in wave_blocks[:PREFETCH]:
                            gtiles[blk[0]] = emit_gather(blk)
                        pending = wave_blocks[PREFETCH:]
                    else:
                        # dma_gather reads tab_work: all reads must precede
                        # this wave's scatters in emission order
                        for blk in wave_blocks:
                            gtiles[blk[0]] = emit_gather(blk)
                        pending = []
                    if not copied:
                        copied = True
                        emit_copy()

                g = gtiles.pop(cs)
                if pending:
                    blk = pending.pop(0)
                    gtiles[blk[0]] = emit_gather(blk)
                dhb = dhpool.tile([128, 2 * BLOCK, S], f32, tag="dh",
                                  name=f"dh_{cs}")
                for q in range(nb):
                    c = cs + q
                    hg2 = g[:, 2 * q:2 * q + 2, :].rearrange("p a b -> p (a b)")

                    # PE transpose of H in f32; the PSUM->SBUF copy casts bf16
                    ht_ps = psA.tile([128, 4, CHUNK], f32, tag="tr")
                    for k in range(4):
                        nc.tensor.transpose(
                            ht_ps[:, k, :], hg2[:, CHUNK * k:CHUNK * (k + 1)],
                            ident[:])
                    ht = wpool.tile([128, 4, CHUNK], bf16, tag="ht")
                    nc.vector.tensor_copy(ht[:], ht_ps[:])

                    xt_c = xT_sb[:, CHUNK * c:CHUNK * (c + 1)]

                    zpre = psZ.tile([128, S2], f32, tag="zpre")
                    rpre = psR.tile([128, S2], f32, tag="rpre")
                    # interleave z/r accumulation groups: alternating PSUM
                    # banks hides any per-bank accumulate bubble
                    nc.tensor.matmul(zpre[:], xt_c, w_sb["WzT"][:],
                                     start=True, stop=False)
                    nc.tensor.matmul(rpre[:], xt_c, w_sb["WrT"][:],
                                     start=True, stop=False)
                    for k in range(4):
                        nc.tensor.matmul(zpre[:], ht[:, k, :], w_sb["UzT"][:, k, :],
                                         start=False, stop=(k == 3))
                        nc.tensor.matmul(rpre[:], ht[:, k, :], w_sb["UrT"][:, k, :],
                                         start=False, stop=(k == 3))

                    zc = wpool.tile([128, S2], f32, tag="zc")
                    r = wpool.tile([128, S2], f32, tag="r")
                    nc.scalar.activation(zc[:], zpre[:], Sig, scale=-1.0)  # 1-z
                    nc.scalar.activation(r[:], rpre[:], Sig)

                    rh = wpool.tile([128, S2], bf16, tag="rh")
                    nc.vector.tensor_mul(rh[:], r[:], hg2)
                    rht_ps_f = psA.tile([128, 4, CHUNK], f32, tag="tr",
                                        name=f"rhtp_{c}")
                    rht_ps = rht_ps_f[:].bitcast(bf16)[:, :, 0:CHUNK]
                    for k in range(4):
                        nc.tensor.transpose(
                            rht_ps[:, k, :], rh[:, CHUNK * k:CHUNK * (k + 1)],
                            identb[:])
                    rht = wpool.tile([128, 4, CHUNK], bf16, tag="rht")
                    nc.vector.tensor_copy(rht[:], rht_ps)

                    mpre = psM.tile([128, S2], f32, tag="mpre")
                    nc.tensor.matmul(mpre[:], xt_c, w_sb["WhT"][:],
                                     start=True, stop=False)
                    for k in range(4):
                        nc.tensor.matmul(mpre[:], rht[:, k, :], w_sb["UhT"][:, k, :],
                                         start=False, stop=(k == 3))

                    m = wpool.tile([128, S2], f32, tag="m")
                    nc.scalar.activation(m[:], mpre[:], Tanh)

                    # dh = (1-z)*(m-h)
                    t1 = wpool.tile([128, S2], f32, tag="t1")
                    nc.vector.tensor_sub(t1[:], m[:], hg2)
                    dh_view = dhb[:, 2 * q:2 * (q + 1), :].rearrange(
                        "p a b -> p (a b)")
                    nc.vector.tensor_mul(dh_view, zc[:], t1[:])
                    if hd['dup_any']:
                        tm = wpool.tile([128, S], f32, tag="tm")
                        nc.vector.tensor_scalar_mul(
                            tm[:], dhb[:, 2 * q + 1, :], dmask_sb[:, c:c + 1])
                        nc.vector.tensor_add(
                            dhb[:, 2 * q, :], dhb[:, 2 * q, :], tm[:])

                # ship deltas to host (sync HWDGE)
                nc.sync.dma_start(dh_out[:, 2 * cs:2 * (cs + nb), :],
                                  dhb[:, 0:2 * nb, :])
                if any_keep:
                    nidx = 2 * CHUNK * nb
                    nc.gpsimd.dma_scatter_add(
                        tab_work[:], dhb[:, 0:2 * nb, :],
                        idx_sb[:, 16 * cs:16 * cs + nidx // 16],
                        nidx, nidx, S, queue_num=0,
                    )

    nc.compile()
    return nc


def _in_map(hd):
    dmask_full = np.zeros((128, hd['n_chunks']), dtype=np.float32)
    dmask_full[:] = hd['dupmask'].reshape(hd['n_chunks'], CHUNK).T
    return {
        "table": hd['table_c'], "idx": hd['idx_rep'], "fresh": hd['fresh_arr'],
        "xT": hd['xT'], "WzT": hd['WzT'], "WrT": hd['WrT'], "WhT": hd['WhT'],
        "UzT": hd['UzT'], "UrT": hd['UrT'], "UhT": hd['UhT'],
        "dmask": dmask_full,
    }


def _run(hd, nc, trace=False):
    from concourse.bass_utils import run_bass_kernel_spmd
    in_map = _in_map(hd)
    return run_bass_kernel_spmd(nc, [dict(in_map) for _ in range(8)],
                                list(range(8)), trace=trace)


def _assemble(hd, dh_res, table0):
    """Host-side final assembly: ordered scatter-add of all deltas."""
    n_chunks = hd['n_chunks']
    # dh_res [128, 2*n_chunks, 256] -> per step i=c*128+p, slot s: [p, 2c+s, :]
    dh = np.ascontiguousarray(dh_res.transpose(1, 0, 2))  # [2c+s, p, 256]
    dh = dh.reshape(n_chunks, 2, CHUNK, S).transpose(0, 2, 1, 3)
    dh = dh.reshape(hd['T_pad'] * 2, S)  # [(step, slot), 256] schedule order
    rows = np.stack([hd['b_s'], hd['p_s']], axis=1).reshape(-1)
    acc = hd['table_c'].copy()
    np.add.at(acc, rows, dh)
    out = table0.copy()
    out[hd['touched']] = acc[:hd['n_real']]
    return out


def kernel(**inputs):
    x = np.asarray(inputs['x'], dtype=np.float32)
    b = np.asarray(inputs['b'])
    p = np.asarray(inputs['p'])
    table0 = np.asarray(inputs['table0'], dtype=np.float32)

    hd = _build_host_data(
        x, b, p,
        np.asarray(inputs['Wz'], np.float32), np.asarray(inputs['Wr'], np.float32),
        np.asarray(inputs['Wh'], np.float32), np.asarray(inputs['Uz'], np.float32),
        np.asarray(inputs['Ur'], np.float32), np.asarray(inputs['Uh'], np.float32),
        np.asarray(inputs['bz'], np.float32), np.asarray(inputs['br'], np.float32),
        np.asarray(inputs['bh'], np.float32), table0)

    nc = _build_nc(hd)
    res = _run(hd, nc)
    return _assemble(hd, np.asarray(res.results[0]["dh"], np.float32), table0)


if __name__ == "__main__":
    d = np.load('/tmp/ref_inputs.npz')
    inputs = {k: d[k] for k in d.files}
    got = kernel(**inputs)
    exp = np.load('/tmp/ref_out_np.npy')
    err = np.abs(got - exp).max()
    print("abs err:", err, "rel:", err / np.abs(exp).max())
